# revision 2
# baseline (speedup 1.0000x reference)
"""Bass kernel v2 for nn_ContextualAttention on 8 trn2 cores.

Sharding: core = 2*s + q (s = sample 0..3, q = lf-half 0..1), as baseline.
Changes vs baseline:
- pass-1/pass-2 diagonal-fuse moved off the tensor engine: partition-shifted
  SBUF->SBUF DMA snapshots + DVE adds (saves ~116us of PE time).
- recon restructured into 4 output-parity classes: the 4 taps of a class
  accumulate in PSUM (rhs column shifts on E), one strided drain per chunk;
  removes the serialized gpsimd accumulate chain and ~8% of recon rows.
- 1/Z (and /4) folded into E before recon matmuls.
- initial DMAs reordered (fpt on scalar queue, wt first on sync) to cut the
  ~25us start gap.
"""
import numpy as np
import ml_dtypes
import contextlib
import concourse.bass as bass
from concourse import bacc, bass_isa
import concourse.tile as tile
from concourse import mybir

F16 = mybir.dt.float16
F32 = mybir.dt.float32
BF16 = mybir.dt.bfloat16
AL = mybir.AluOpType
AF = mybir.ActivationFunctionType

G = 48
J = 18
KT = 9
LB = 2304
C = 128
WINP = 30
WIN = WINP * G          # 1440
FT0 = WIN               # slab col of far_top block
FB0 = WIN + 96          # slab col of far_bot block
NCOL = WIN + 192        # 1632
CONS0 = 2 * G           # 96
NA = 26 * G             # 1248
SCH = 408               # scores matmul N-chunk (4 per NCOL)
P1W = 192               # pass-1 window chunk (7 per 1344)
P2C = 156               # pass-2 chunk (8 per NA)
ESC_BIAS = 1152 * 1e-4


def _pshift(nc, eng, dst, src_region, zc, w):
    """dst[p,j,:] = src[p+1,j,:] chained over j; dst[127,J-1,:] = 0.
    src_region: AP [128, J, w] (a column slice of a [128, J, *] tile)."""
    eng.dma_start(out=dst[0:127, :, 0:w], in_=src_region[1:128, :, :])
    eng.dma_start(out=dst[127:128, 0:J - 1, 0:w], in_=src_region[0:1, 1:J, :])
    eng.dma_start(out=dst[127:128, J - 1, 0:w], in_=zc[0:1, 0:w])


def _mshift(nc, eng, dst, src_region, zc, w):
    """dst[p,j,:] = src[p-1,j,:] chained over j; dst[0,0,:] = 0."""
    eng.dma_start(out=dst[1:128, :, 0:w], in_=src_region[0:127, :, :])
    eng.dma_start(out=dst[0:1, 1:J, 0:w], in_=src_region[127:128, 0:J - 1, :])
    eng.dma_start(out=dst[0:1, 0, 0:w], in_=zc[0:1, 0:w])


def _p48shift(nc, eng, dst, src_region, zc, w):
    """dst[p,j] = src[p+48, j] with j-chain; j=J-1 tail per CW map."""
    eng.dma_start(out=dst[0:80, :, 0:w], in_=src_region[48:128, :, :])
    eng.dma_start(out=dst[80:128, 0:J - 1, 0:w], in_=src_region[0:48, 1:J, :])
    eng.dma_start(out=dst[80:127, J - 1, 0:w], in_=src_region[1:48, 0, :])
    eng.dma_start(out=dst[127:128, J - 1, 0:w], in_=zc[0:1, 0:w])


def _m48shift(nc, eng, dst, src_region, zc, w):
    """dst[p,j] = src[p-48, j] with j-chain; j=0 head per CPW map."""
    eng.dma_start(out=dst[48:128, :, 0:w], in_=src_region[0:80, :, :])
    eng.dma_start(out=dst[0:48, 1:J, 0:w], in_=src_region[80:128, 0:J - 1, :])
    eng.dma_start(out=dst[1:48, 0, 0:w], in_=src_region[80:127, J - 1, :])
    eng.dma_start(out=dst[0:1, 0, 0:w], in_=zc[0:1, 0:w])


def build(debug=False):
    nc = bacc.Bacc()
    fp_d = nc.dram_tensor("fp", [KT, 128, NCOL], F16, kind="ExternalInput")
    wt_d = nc.dram_tensor("wt", [J, KT, 128, 128], F16, kind="ExternalInput")
    rden_d = nc.dram_tensor("rden", [128, J], F32, kind="ExternalInput")
    rawt_d = nc.dram_tensor("rawt", [16, J, 128, 128], BF16, kind="ExternalInput")
    s10_d = nc.dram_tensor("s10", [128, J], F32, kind="ExternalInput")
    mbin_d = nc.dram_tensor("mbin", [128, J], F32, kind="ExternalInput")
    gate_d = nc.dram_tensor("gate", [128, 2], F32, kind="ExternalInput")
    out_d = nc.dram_tensor("out", [128, 48, 96], F32, kind="ExternalOutput")

    with tile.TileContext(nc) as tc, contextlib.ExitStack() as ctx:
        consts = ctx.enter_context(tc.tile_pool(name="consts", bufs=1))
        wtp = ctx.enter_context(tc.tile_pool(name="wtp", bufs=4))
        big = ctx.enter_context(tc.tile_pool(name="big", bufs=1))
        work = ctx.enter_context(tc.tile_pool(name="work", bufs=2))

        # ---------------- consts (gpsimd queue; tiny) ----------------
        s10t = consts.tile([128, J], F32, tag="s10t")
        nc.gpsimd.dma_start(out=s10t, in_=s10_d[:, :])
        mbint = consts.tile([128, J], F32, tag="mbint")
        nc.gpsimd.dma_start(out=mbint, in_=mbin_d[:, :])
        gatet = consts.tile([128, 2], F32, tag="gatet")
        nc.gpsimd.dma_start(out=gatet, in_=gate_d[:, :])
        rdent = consts.tile([128, J], F32, tag="rdent")
        nc.gpsimd.dma_start(out=rdent, in_=rden_d[:, :])
        ones16 = consts.tile([128, 1], BF16, tag="ones16")
        nc.vector.memset(ones16, 1.0)
        zc = consts.tile([128, P1W], F16, tag="zc")
        nc.vector.memset(zc, 0.0)
        Zrow = consts.tile([1, NA], F32, tag="Zrow")
        rzrow = consts.tile([1, NA], F32, tag="rzrow")
        rzb = consts.tile([128, NA], F32, tag="rzb")

        # ---------------- scores GEMM -> slab (Sn, fp16) ----------------
        # wt j=0 goes first on sync; fpt streams on scalar queue.
        fpt = big.tile([128, KT, NCOL], F16, tag="U1")
        slab = big.tile([128, J, NCOL], F16, tag="slab")
        for o in range(KT):
            nc.scalar.dma_start(out=fpt[:, o, :], in_=fp_d[o, :, :])
        with tc.tile_pool(name="psc", bufs=2, space="PSUM") as psc:
            for j in range(J):
                ps = psc.tile([128, 4, 512], F32, tag="sps")
                for o in range(KT):
                    wb = wtp.tile([128, 128], F16, tag="wb")
                    eng = nc.sync if (o % 2 == 0) else nc.gpsimd
                    eng.dma_start(out=wb, in_=wt_d[j, o, :, :])
                    for ch in range(4):
                        nc.tensor.matmul(ps[:, ch, 0:SCH], wb,
                                         fpt[:, o, ch * SCH:(ch + 1) * SCH],
                                         start=(o == 0), stop=(o == KT - 1))
                for ch in range(4):
                    nc.scalar.activation(slab[:, j, ch * SCH:(ch + 1) * SCH],
                                         ps[:, ch, 0:SCH], AF.Copy,
                                         scale=rdent[:, j:j + 1])

        # ---------------- pass-1: S1 = Sn + diag(+1) + diag(-1) ----------------
        # S1 layout as baseline: local l = global-48 for window, far_top at
        # [1344,1440) <- slab[1440,1536), far_bot [1440,1536) <- slab[1536,1632).
        S1 = big.tile([128, J, 1536], F16, tag="U1")
        with tc.tile_pool(name="shift", bufs=6) as shp:
            for c0 in range(0, 1344, P1W):
                Ach = shp.tile([128, J, P1W], F16, tag="sh")
                _pshift(nc, nc.sync, Ach, slab[:, :, 49 + c0:49 + c0 + P1W], zc, P1W)
                Mch = shp.tile([128, J, P1W], F16, tag="sh")
                _mshift(nc, nc.sync, Mch, slab[:, :, 47 + c0:47 + c0 + P1W], zc, P1W)
                nc.vector.tensor_tensor(out=S1[:, :, c0:c0 + P1W],
                                        in0=slab[:, :, 48 + c0:48 + c0 + P1W],
                                        in1=Ach, op=AL.add)
                nc.vector.tensor_tensor(out=S1[:, :, c0:c0 + P1W],
                                        in0=S1[:, :, c0:c0 + P1W],
                                        in1=Mch, op=AL.add)
            for f0, l0 in ((FT0, 1344), (FB0, 1440)):
                nc.sync.dma_start(out=S1[:, :, l0:l0 + 96], in_=slab[:, :, f0:f0 + 96])
                Af = shp.tile([128, J, 95], F16, tag="sh")
                _pshift(nc, nc.sync, Af, slab[:, :, f0 + 1:f0 + 96], zc, 95)
                Mf = shp.tile([128, J, 95], F16, tag="sh")
                _mshift(nc, nc.sync, Mf, slab[:, :, f0:f0 + 95], zc, 95)
                nc.vector.tensor_tensor(out=S1[:, :, l0:l0 + 95],
                                        in0=S1[:, :, l0:l0 + 95],
                                        in1=Af[:, :, 0:95], op=AL.add)
                nc.vector.tensor_tensor(out=S1[:, :, l0 + 1:l0 + 96],
                                        in0=S1[:, :, l0 + 1:l0 + 96],
                                        in1=Mf[:, :, 0:95], op=AL.add)
            # gates: zero pi=2 block (q=0) / pi=27 block (q=1)
            nc.vector.tensor_scalar_mul(S1[:, :, 48:96], S1[:, :, 48:96], gatet[:, 0:1])
            nc.vector.tensor_scalar_mul(S1[:, :, 1248:1296], S1[:, :, 1248:1296], gatet[:, 1:2])

            # ---------------- pass-2 + softmax (E overlays slab) ----------------
            Ebig = slab[:, :, 0:NA].bitcast(BF16)
            s10bc = bass.AP(tensor=s10t.tensor, offset=s10t.offset,
                            ap=[s10t.ap[0], [1, J], [0, P2C]])
            mbv = bass.AP(tensor=mbint.tensor, offset=mbint.offset,
                          ap=[mbint.ap[0], [1, J], [0, P2C]])
            with tc.tile_pool(name="psz", bufs=2, space="PSUM") as psz:
                for ci in range(NA // P2C):
                    d0 = ci * P2C
                    l0 = 48 + d0
                    A48 = shp.tile([128, J, P2C], F16, tag="sh")
                    _p48shift(nc, nc.sync, A48, S1[:, :, l0 + 48:l0 + 48 + P2C], zc, P2C)
                    M48 = shp.tile([128, J, P2C], F16, tag="sh")
                    _m48shift(nc, nc.sync, M48, S1[:, :, d0:d0 + P2C], zc, P2C)
                    S2 = work.tile([128, J, P2C], F32, tag="S2")
                    nc.vector.tensor_tensor(out=S2, in0=S1[:, :, l0:l0 + P2C],
                                            in1=A48[:, :, 0:P2C], op=AL.add)
                    nc.vector.tensor_tensor(out=S2, in0=S2, in1=M48[:, :, 0:P2C], op=AL.add)
                    if ci == 0:
                        Mf48 = shp.tile([128, J, 47], F16, tag="sh")
                        _m48shift(nc, nc.sync, Mf48, S1[:, :, 1488:1535], zc, 47)
                        nc.vector.tensor_tensor(out=S2[:, :, 49:96], in0=S2[:, :, 49:96],
                                                in1=Mf48[:, :, 0:47], op=AL.add)
                    if ci == 7:
                        Af48 = shp.tile([128, J, 47], F16, tag="sh")
                        _p48shift(nc, nc.sync, Af48, S1[:, :, 1345:1392], zc, 47)
                        nc.vector.tensor_tensor(out=S2[:, :, 60:107], in0=S2[:, :, 60:107],
                                                in1=Af48[:, :, 0:47], op=AL.add)
                    nc.vector.tensor_tensor(out=S2, in0=S2, in1=s10bc, op=AL.mult)
                    # max over lb
                    t9 = work.tile([128, 9, P2C], BF16, tag="tA")
                    nc.vector.tensor_tensor(out=t9, in0=S2[:, 0:9, :], in1=S2[:, 9:18, :], op=AL.max)
                    t4 = work.tile([128, 4, P2C], BF16, tag="tB")
                    nc.vector.tensor_tensor(out=t4, in0=t9[:, 0:4, :], in1=t9[:, 4:8, :], op=AL.max)
                    t2 = work.tile([128, 2, P2C], BF16, tag="t2")
                    nc.vector.tensor_tensor(out=t2, in0=t4[:, 0:2, :], in1=t4[:, 2:4, :], op=AL.max)
                    mx = work.tile([128, P2C], BF16, tag="mx")
                    nc.vector.tensor_tensor(out=mx, in0=t2[:, 0, :], in1=t2[:, 1, :], op=AL.max)
                    nc.vector.tensor_tensor(out=mx, in0=mx, in1=t9[:, 8, :], op=AL.max)
                    mxb = work.tile([128, P2C], F32, tag="mxb")
                    nc.gpsimd.partition_all_reduce(mxb, mx, channels=128,
                                                   reduce_op=bass_isa.ReduceOp.max)
                    mview = bass.AP(tensor=mxb.tensor, offset=mxb.offset,
                                    ap=[mxb.ap[0], [0, J], mxb.ap[1]])
                    nc.vector.tensor_tensor(out=S2, in0=S2, in1=mview, op=AL.subtract)
                    # E = exp(u) -> bf16 overlay
                    nc.scalar.activation(Ebig[:, :, d0:d0 + P2C], S2, AF.Exp)
                    # Z = ones^T E (before masking, as reference)
                    zp = psz.tile([1, P2C], F32, tag="zp")
                    for j in range(J):
                        nc.tensor.matmul(zp, ones16, Ebig[:, j, d0:d0 + P2C],
                                         start=(j == 0), stop=(j == J - 1))
                    nc.scalar.activation(Zrow[:, d0:d0 + P2C], zp, AF.Copy)
                    # E *= mask rows; E *= 0.25/Z (fold recon normalization)
                    nc.vector.tensor_tensor(out=Ebig[:, :, d0:d0 + P2C],
                                            in0=Ebig[:, :, d0:d0 + P2C], in1=mbv, op=AL.mult)
                    nc.vector.reciprocal(rzrow[:, d0:d0 + P2C], Zrow[:, d0:d0 + P2C])
                    nc.vector.tensor_scalar_mul(rzrow[:, d0:d0 + P2C], rzrow[:, d0:d0 + P2C], 0.25)
                    nc.gpsimd.partition_broadcast(rzb[:, d0:d0 + P2C], rzrow[:, d0:d0 + P2C])
                    rzv = bass.AP(tensor=rzb.tensor, offset=rzb.offset + d0,
                                  ap=[rzb.ap[0], [0, J], [1, P2C]])
                    nc.vector.tensor_tensor(out=Ebig[:, :, d0:d0 + P2C],
                                            in0=Ebig[:, :, d0:d0 + P2C], in1=rzv, op=AL.mult)
        # E gates (phantom att cols)
        nc.vector.tensor_scalar_mul(Ebig[:, :, 0:G], Ebig[:, :, 0:G], gatet[:, 0:1])
        nc.vector.tensor_scalar_mul(Ebig[:, :, NA - G:NA], Ebig[:, :, NA - G:NA], gatet[:, 1:2])

        # ---------------- recon: 4 output-parity classes ----------------
        # out(y=2*yi+ey, x=2*xi+ex) = sum over 4 taps (ky,kx) of class:
        #   raw[tap]^T E'[d], d = (yi+piofs-2)*48 + xi + uofs
        out_acc = big.tile([128, 48, 96], F32, tag="U1")
        with tc.tile_pool(name="psg", bufs=6, space="PSUM") as psg, \
                tc.tile_pool(name="rawp", bufs=2) as rawp:
            for ey in (0, 1):
                for ex in (0, 1):
                    kys = (1, 3) if ey == 0 else (2, 0)
                    kxs = (1, 3) if ex == 0 else (2, 0)
                    # (ky, kx, piofs, uofs); full-width (uofs=0) taps first
                    taps = []
                    for kx in kxs:
                        for ky in kys:
                            piofs = 3 - (ky - 1 - ey) // 2
                            uofs = -(kx - 1 - ex) // 2
                            taps.append((ky, kx, piofs, uofs))
                    raw4 = rawp.tile([128, 4, J, 128], BF16, tag="r4")
                    for ti, (ky, kx, _, _) in enumerate(taps):
                        tap = ky * 4 + kx
                        rin = bass.AP(tensor=rawt_d.ap().tensor, offset=tap * J * 128 * 128,
                                      ap=[[128, 128], [128 * 128, J], [1, 128]])
                        nc.scalar.dma_start(out=raw4[:, ti, :, :], in_=rin)
                    for rc in range(3):
                        yi0 = 8 * rc
                        gp = psg.tile([128, 384], F32, tag="gp")
                        nmm = 0
                        for j in range(J):
                            for ti, (ky, kx, piofs, uofs) in enumerate(taps):
                                rowbase = (yi0 + piofs - 2) * G
                                if uofs == 0:
                                    rhs = Ebig[:, j, rowbase:rowbase + 384]
                                    outv = gp[:, 0:384]
                                else:
                                    xi_lo = 1 if uofs == -1 else 0
                                    rhs = bass.AP(tensor=Ebig.tensor,
                                                  offset=Ebig.offset + j * NCOL + rowbase + xi_lo + uofs,
                                                  ap=[Ebig.ap[0], [G, 8], [1, 47]])
                                    outv = bass.AP(tensor=gp.tensor, offset=gp.offset + xi_lo,
                                                   ap=[gp.ap[0], [G, 8], [1, 47]])
                                nmm += 1
                                nc.tensor.matmul(outv, raw4[:, ti, j, :], rhs,
                                                 start=(nmm == 1), stop=(nmm == 4 * J))
                        oview = bass.AP(tensor=out_acc.tensor,
                                        offset=out_acc.offset + (2 * yi0 + ey) * 96 + ex,
                                        ap=[out_acc.ap[0], [192, 8], [2, 48]])
                        nc.scalar.activation(oview, gp[:, 0:384], AF.Copy)
        nc.sync.dma_start(out=out_d[:, :, :], in_=out_acc)
    nc.finalize()
    return nc


# ======================= host side =======================

def prep_core_inputs(f, b, mask):
    """Full inputs -> list of 8 in_map dicts (core = 2*s + q)."""
    B = f.shape[0]
    ms = np.pad(mask[0][:, ::8, ::8][0], 1)
    w = np.lib.stride_tricks.sliding_window_view(ms, (3, 3))
    mm = (w.sum((2, 3)) == 0).astype(np.float32).reshape(LB)
    s10 = np.ascontiguousarray((10.0 * mm).reshape(J, 128).T)
    mbin = np.ascontiguousarray(mm.reshape(J, 128).T)
    in_maps = []
    for s in range(B):
        fs = f[s][:, ::2, ::2]
        bs = b[s][:, ::2, ::2]
        fsp = np.pad(fs, ((0, 0), (1, 1), (1, 1)))
        bsp = np.pad(bs, ((0, 0), (1, 1), (1, 1)))
        bhwc = np.pad(b[s], ((0, 0), (1, 1), (1, 1))).transpose(1, 2, 0)
        Q = (bsp.astype(np.float32) ** 2).sum(0).reshape(2500)
        A = Q.copy(); A[0:2499] += Q[1:2500]; A[1:2500] += Q[0:2499]
        Bb = A.copy(); Bb[0:2450] += A[50:2500]; Bb[50:2500] += A[0:2450]
        win = np.lib.stride_tricks.as_strided(
            Bb[51:], shape=(48, 48), strides=(Bb.strides[0] * 50, Bb.strides[0]))
        rd = (1.0 / np.sqrt(win.reshape(LB) + ESC_BIAS)).astype(np.float32)
        rden = np.ascontiguousarray(rd.reshape(J, 128).T)
        wt = np.empty((KT, C, LB), np.float32)
        for o in range(KT):
            dy, dx = o // 3, o % 3
            wt[o] = bsp[:, dy:dy + G, dx:dx + G].reshape(C, LB)
        wt_blocks = np.ascontiguousarray(
            wt.reshape(KT, C, J, 128).transpose(2, 0, 1, 3)).astype(np.float16)
        iy, ix = np.divmod(np.arange(LB), G)
        rawt = np.empty((16, LB, C), np.float32)
        for ky in range(4):
            for kx in range(4):
                rawt[ky * 4 + kx] = bhwc[2 * iy + ky, 2 * ix + kx, :]
        rawt = np.ascontiguousarray(rawt.reshape(16, J, 128, C)).astype(ml_dtypes.bfloat16)
        for q in (0, 1):
            ts_ = np.arange(WINP) - 3 + 24 * q
            fcols = np.zeros((KT, C, NCOL), np.float32)
            valid = (ts_ >= 0) & (ts_ < G)
            for o in range(KT):
                dy, dx = o // 3, o % 3
                block = fsp[:, (ts_ + dy).clip(0, G + 1), :][:, :, dx:dx + G]
                block = block * valid[None, :, None]
                fcols[o, :, :WIN] = block.reshape(C, WIN)
                if q == 1:
                    fcols[o, :, FT0:FT0 + 96] = fsp[:, dy:dy + 2, dx:dx + G].reshape(C, 96)
                else:
                    fcols[o, :, FB0:FB0 + 96] = fsp[:, 46 + dy:48 + dy, dx:dx + G].reshape(C, 96)
            gate = np.zeros((128, 2), np.float32)
            gate[:, 0] = 0.0 if q == 0 else 1.0
            gate[:, 1] = 1.0 if q == 0 else 0.0
            in_maps.append(dict(
                fp=fcols.astype(np.float16),
                wt=wt_blocks,
                rden=rden,
                rawt=rawt,
                s10=s10, mbin=mbin, gate=gate,
            ))
    return in_maps


def assemble(results, B=4):
    out = np.zeros((B, C, 96, 96), np.float32)
    for s in range(B):
        for q in (0, 1):
            out[s, :, 48 * q:48 * q + 48, :] = results[2 * s + q]["out"]
    return out


# ======================= self-contained runner =======================
_NC_CACHE = {}
last_exec_time_ns = None


def kernel(f, b, mask):
    global last_exec_time_ns
    import os
    from concourse.bass_utils import run_bass_kernel_spmd
    f = np.ascontiguousarray(np.asarray(f, dtype=np.float32))
    b = np.ascontiguousarray(np.asarray(b, dtype=np.float32))
    mask = np.ascontiguousarray(np.asarray(mask, dtype=np.float32))
    in_maps = prep_core_inputs(f, b, mask)
    if "nc" not in _NC_CACHE:
        _NC_CACHE["nc"] = build(debug=False)
    nc = _NC_CACHE["nc"]
    trace = bool(os.environ.get("BASS_TRACE"))
    tmpdir = os.environ.get("BASS_TMPDIR") or None
    res = run_bass_kernel_spmd(nc, in_maps, core_ids=list(range(8)), trace=trace,
                               tmpdir=tmpdir)
    last_exec_time_ns = res.exec_time_ns
    return assemble([res.results[i] for i in range(8)], B=f.shape[0])


# revision 3
# speedup vs baseline: 2.1384x; 2.1384x over previous
"""Bass kernel v2 for nn_ContextualAttention on 8 trn2 cores.

Sharding: core = 2*s + q (s = sample 0..3, q = lf-half 0..1), as baseline.
Changes vs baseline:
- pass-1/pass-2 diagonal-fuse moved off the tensor engine: partition-shifted
  SBUF->SBUF DMA snapshots + DVE adds (saves ~116us of PE time).
- recon restructured into 4 output-parity classes: the 4 taps of a class
  accumulate in PSUM (rhs column shifts on E), one strided drain per chunk;
  removes the serialized gpsimd accumulate chain and ~8% of recon rows.
- 1/Z (and /4) folded into E before recon matmuls.
- initial DMAs reordered (fpt on scalar queue, wt first on sync) to cut the
  ~25us start gap.
"""
import numpy as np
import ml_dtypes
import contextlib
import concourse.bass as bass
from concourse import bacc, bass_isa
import concourse.tile as tile
from concourse import mybir

F16 = mybir.dt.float16
F32 = mybir.dt.float32
BF16 = mybir.dt.bfloat16
AL = mybir.AluOpType
AF = mybir.ActivationFunctionType

G = 48
J = 18
KT = 9
LB = 2304
C = 128
WINP = 30
WIN = WINP * G          # 1440
FT0 = WIN               # slab col of far_top block
FB0 = WIN + 96          # slab col of far_bot block
NCOL = WIN + 192        # 1632
CONS0 = 2 * G           # 96
NA = 26 * G             # 1248
SCH = 408               # scores matmul N-chunk (4 per NCOL)
P1W = 192               # pass-1 window chunk (7 per 1344)
P2C = 156               # pass-2 chunk (8 per NA)
ESC_BIAS = 1152 * 1e-4


def _pshift_half(nc, engs, dst, slab, js, je, zc):
    """dst[p,jj,:] = slab[p+1, js+jj, :] chained over global j; full NCOL width.
    Main copy split into 4 partition ranges across queues so DMA channels
    parallelize; every run is per-partition contiguous (1 descriptor each)."""
    nj = je - js
    for qi, (p0, p1) in enumerate(((0, 32), (32, 64), (64, 96), (96, 127))):
        engs[qi % len(engs)].dma_start(out=dst[p0:p1, 0:nj, :],
                                       in_=slab[p0 + 1:p1 + 1, js:je, :])
    if je < J:
        engs[0].dma_start(out=dst[127:128, 0:nj, :], in_=slab[0:1, js + 1:je + 1, :])
    else:
        if nj > 1:
            engs[0].dma_start(out=dst[127:128, 0:nj - 1, :], in_=slab[0:1, js + 1:J, :])
        engs[1 % len(engs)].dma_start(out=dst[127:128, nj - 1, :], in_=zc[0:1, 0:NCOL])


def _mshift_half(nc, engs, dst, slab, js, je, zc):
    """dst[p,jj,:] = slab[p-1, js+jj, :] chained over global j."""
    nj = je - js
    for qi, (p0, p1) in enumerate(((1, 33), (33, 65), (65, 97), (97, 128))):
        engs[qi % len(engs)].dma_start(out=dst[p0:p1, 0:nj, :],
                                       in_=slab[p0 - 1:p1 - 1, js:je, :])
    if js > 0:
        engs[0].dma_start(out=dst[0:1, 0:nj, :], in_=slab[127:128, js - 1:je - 1, :])
    else:
        if nj > 1:
            engs[0].dma_start(out=dst[0:1, 1:nj, :], in_=slab[127:128, 0:nj - 1, :])
        engs[1 % len(engs)].dma_start(out=dst[0:1, 0, :], in_=zc[0:1, 0:NCOL])


def build(debug=False):
    nc = bacc.Bacc()
    fp_d = nc.dram_tensor("fp", [KT, 128, NCOL], F16, kind="ExternalInput")
    wt_d = nc.dram_tensor("wt", [J, KT, 128, 128], F16, kind="ExternalInput")
    rden_d = nc.dram_tensor("rden", [128, J], F32, kind="ExternalInput")
    shm_d = nc.dram_tensor("shm", [128, 11, 128], F16, kind="ExternalInput")
    rawt_d = nc.dram_tensor("rawt", [16, J, 128, 128], BF16, kind="ExternalInput")
    s10_d = nc.dram_tensor("s10", [128, J], F32, kind="ExternalInput")
    mbin_d = nc.dram_tensor("mbin", [128, J], F32, kind="ExternalInput")
    gate_d = nc.dram_tensor("gate", [128, 2], F32, kind="ExternalInput")
    out_d = nc.dram_tensor("out", [128, 48, 96], F32, kind="ExternalOutput")

    with tile.TileContext(nc) as tc, contextlib.ExitStack() as ctx:
        consts = ctx.enter_context(tc.tile_pool(name="consts", bufs=1))
        wtp = ctx.enter_context(tc.tile_pool(name="wtp", bufs=4))
        big = ctx.enter_context(tc.tile_pool(name="big", bufs=1))
        work = ctx.enter_context(tc.tile_pool(name="work", bufs=2))

        # ---------------- consts (gpsimd queue; tiny) ----------------
        s10t = consts.tile([128, J], F32, tag="s10t")
        nc.gpsimd.dma_start(out=s10t, in_=s10_d[:, :])
        mbint = consts.tile([128, J], F32, tag="mbint")
        nc.gpsimd.dma_start(out=mbint, in_=mbin_d[:, :])
        gatet = consts.tile([128, 2], F32, tag="gatet")
        nc.gpsimd.dma_start(out=gatet, in_=gate_d[:, :])
        rdent = consts.tile([128, J], F32, tag="rdent")
        nc.gpsimd.dma_start(out=rdent, in_=rden_d[:, :])
        ones16 = consts.tile([128, 1], BF16, tag="ones16")
        nc.vector.memset(ones16, 1.0)
        zc = consts.tile([128, NCOL], F16, tag="zc")
        nc.vector.memset(zc, 0.0)
        # shift matrices for pass-2 matmuls:
        # [I, ., ., ., ., Ap48, Bp48, Cw, Am48, Bm48, Cpw]
        shmt = consts.tile([128, 11, 128], F16, tag="shmt")
        nc.gpsimd.dma_start(out=shmt, in_=shm_d[:, :, :])

        # ---------------- scores GEMM -> slab (Sn, fp16) ----------------
        # wt j=0 goes first on sync; fpt streams on scalar queue.
        fpt = big.tile([128, KT, NCOL], F16, tag="U1")
        slab = big.tile([128, J, NCOL], F16, tag="slab")
        for o in range(KT):
            nc.scalar.dma_start(out=fpt[:, o, :], in_=fp_d[o, :, :])
        with tc.tile_pool(name="psc", bufs=2, space="PSUM") as psc:
            for j in range(J):
                ps = psc.tile([128, 4, 512], F32, tag="sps")
                for o in range(KT):
                    wb = wtp.tile([128, 128], F16, tag="wb")
                    eng = nc.sync if (o % 2 == 0) else nc.gpsimd
                    eng.dma_start(out=wb, in_=wt_d[j, o, :, :])
                    for ch in range(4):
                        nc.tensor.matmul(ps[:, ch, 0:SCH], wb,
                                         fpt[:, o, ch * SCH:(ch + 1) * SCH],
                                         start=(o == 0), stop=(o == KT - 1))
                for ch in range(4):
                    nc.scalar.activation(slab[:, j, ch * SCH:(ch + 1) * SCH],
                                         ps[:, ch, 0:SCH], AF.Copy,
                                         scale=rdent[:, j:j + 1])

        # ---------------- pass-1: S1 = Sn + diag(+1) + diag(-1), on DVE -------
        # S1 layout as baseline: local l = global-48 for window, far_top at
        # [1344,1440) <- slab[1440,1536), far_bot [1440,1536) <- slab[1536,1632).
        # Partition shifts via full-width SBUF->SBUF DMA snapshots, one j-half
        # at a time (per-partition contiguous runs -> 1 descriptor each).
        S1 = big.tile([128, J, 1536], F16, tag="U1")
        engs = [nc.sync, nc.scalar, nc.gpsimd]
        with tc.tile_pool(name="shift", bufs=2) as shp:
            for js in range(0, J, 3):
                je = js + 3
                nj = je - js
                Ah = shp.tile([128, 3, NCOL], F16, tag="shA")
                _pshift_half(nc, engs, Ah, slab, js, je, zc)
                Mh = shp.tile([128, 3, NCOL], F16, tag="shM")
                _mshift_half(nc, engs, Mh, slab, js, je, zc)
                nc.vector.tensor_tensor(out=S1[:, js:je, 0:1344],
                                        in0=slab[:, js:je, 48:1392],
                                        in1=Ah[:, 0:nj, 49:1393], op=AL.add)
                nc.vector.tensor_tensor(out=S1[:, js:je, 0:1344],
                                        in0=S1[:, js:je, 0:1344],
                                        in1=Mh[:, 0:nj, 47:1391], op=AL.add)
                for f0, l0 in ((FT0, 1344), (FB0, 1440)):
                    nc.scalar.activation(S1[:, js:je, l0:l0 + 96],
                                         slab[:, js:je, f0:f0 + 96], AF.Copy)
                    nc.vector.tensor_tensor(out=S1[:, js:je, l0:l0 + 95],
                                            in0=S1[:, js:je, l0:l0 + 95],
                                            in1=Ah[:, 0:nj, f0 + 1:f0 + 96], op=AL.add)
                    nc.vector.tensor_tensor(out=S1[:, js:je, l0 + 1:l0 + 96],
                                            in0=S1[:, js:je, l0 + 1:l0 + 96],
                                            in1=Mh[:, 0:nj, f0:f0 + 95], op=AL.add)
            # gates: zero pi=2 block (q=0) / pi=27 block (q=1)
            nc.vector.tensor_scalar_mul(S1[:, :, 48:96], S1[:, :, 48:96], gatet[:, 0:1])
            nc.vector.tensor_scalar_mul(S1[:, :, 1248:1296], S1[:, :, 1248:1296], gatet[:, 1:2])

        # ---------------- pass-2 (shift matmuls) + softmax (E overlays slab) --
        SH_I = 0
        SH_AP48, SH_BP48, SH_CW, SH_AM48, SH_BM48, SH_CPW = 5, 6, 7, 8, 9, 10
        b_lo, b_hi = 26 * G, 26 * G + G - 1
        bp_lo, bp_hi = 3 * G + 1, 3 * G + G
        Ebig = slab[:, :, 0:NA].bitcast(BF16)
        mbv = bass.AP(tensor=mbint.tensor, offset=mbint.offset,
                      ap=[mbint.ap[0], [1, J], [0, P2C]])
        with tc.tile_pool(name="psp2", bufs=6, space="PSUM") as psp2, \
                tc.tile_pool(name="psz", bufs=2, space="PSUM") as psz:
            for ci in range(NA // P2C):
                d0 = ci * P2C
                g0 = CONS0 + d0
                l0 = g0 - 96
                S2 = work.tile([128, J, P2C], F32, tag="S2")
                for j in range(J):
                    pq = psp2.tile([128, P2C], F32, tag="pq")
                    nc.tensor.matmul(pq, shmt[:, SH_I, :], S1[:, j, l0 + 48:l0 + 48 + P2C],
                                     start=True, stop=False)
                    nc.tensor.matmul(pq, shmt[:, SH_AP48, :], S1[:, j, l0 + 96:l0 + 96 + P2C],
                                     start=False, stop=False)
                    nc.tensor.matmul(pq, shmt[:, SH_AM48, :], S1[:, j, l0:l0 + P2C],
                                     start=False, stop=False)
                    if j < J - 1:
                        nc.tensor.matmul(pq, shmt[:, SH_BP48, :], S1[:, j + 1, l0 + 96:l0 + 96 + P2C],
                                         start=False, stop=False)
                    else:
                        nc.tensor.matmul(pq, shmt[:, SH_CW, :], S1[:, 0, l0 + 96:l0 + 96 + P2C],
                                         start=False, stop=False)
                    last_main = (not (b_lo < g0 + P2C and b_hi > g0)) and (not (bp_lo < g0 + P2C and bp_hi > g0))
                    if j > 0:
                        nc.tensor.matmul(pq, shmt[:, SH_BM48, :], S1[:, j - 1, l0:l0 + P2C],
                                         start=False, stop=last_main)
                    else:
                        nc.tensor.matmul(pq, shmt[:, SH_CPW, :], S1[:, 17, l0:l0 + P2C],
                                         start=False, stop=last_main)
                    if b_lo < g0 + P2C and b_hi > g0:
                        aa = max(b_lo, g0); bb = min(b_hi, g0 + P2C)
                        so = aa - b_lo
                        ft = 1344 + 1 + so
                        oa, nb = aa - g0, bb - aa
                        nc.tensor.matmul(pq[:, oa:oa + nb], shmt[:, SH_AP48, :], S1[:, j, ft:ft + nb],
                                         start=False, stop=False)
                        if j < J - 1:
                            nc.tensor.matmul(pq[:, oa:oa + nb], shmt[:, SH_BP48, :], S1[:, j + 1, ft:ft + nb],
                                             start=False, stop=True)
                        else:
                            nc.tensor.matmul(pq[:, oa:oa + nb], shmt[:, SH_CW, :], S1[:, 0, ft:ft + nb],
                                             start=False, stop=True)
                    if bp_lo < g0 + P2C and bp_hi > g0:
                        aa = max(bp_lo, g0); bb = min(bp_hi, g0 + P2C)
                        so = aa - bp_lo
                        fb = 1440 + 48 + so
                        oa, nb = aa - g0, bb - aa
                        nc.tensor.matmul(pq[:, oa:oa + nb], shmt[:, SH_AM48, :], S1[:, j, fb:fb + nb],
                                         start=False, stop=False)
                        if j > 0:
                            nc.tensor.matmul(pq[:, oa:oa + nb], shmt[:, SH_BM48, :], S1[:, j - 1, fb:fb + nb],
                                             start=False, stop=True)
                        else:
                            nc.tensor.matmul(pq[:, oa:oa + nb], shmt[:, SH_CPW, :], S1[:, 17, fb:fb + nb],
                                             start=False, stop=True)
                    # drain: S2 = pq * s10 (scale applied by activation)
                    nc.scalar.activation(S2[:, j, :], pq, AF.Copy, scale=s10t[:, j:j + 1])
                    # max over lb
                    t9 = work.tile([128, 9, P2C], BF16, tag="tA")
                    nc.vector.tensor_tensor(out=t9, in0=S2[:, 0:9, :], in1=S2[:, 9:18, :], op=AL.max)
                    t4 = work.tile([128, 4, P2C], BF16, tag="tB")
                    nc.vector.tensor_tensor(out=t4, in0=t9[:, 0:4, :], in1=t9[:, 4:8, :], op=AL.max)
                    t2 = work.tile([128, 2, P2C], BF16, tag="t2")
                    nc.vector.tensor_tensor(out=t2, in0=t4[:, 0:2, :], in1=t4[:, 2:4, :], op=AL.max)
                    mx = work.tile([128, P2C], BF16, tag="mx")
                    nc.vector.tensor_tensor(out=mx, in0=t2[:, 0, :], in1=t2[:, 1, :], op=AL.max)
                    nc.vector.tensor_tensor(out=mx, in0=mx, in1=t9[:, 8, :], op=AL.max)
                    mxb = work.tile([128, P2C], F32, tag="mxb")
                    nc.gpsimd.partition_all_reduce(mxb, mx, channels=128,
                                                   reduce_op=bass_isa.ReduceOp.max)
                    mview = bass.AP(tensor=mxb.tensor, offset=mxb.offset,
                                    ap=[mxb.ap[0], [0, J], mxb.ap[1]])
                    nc.vector.tensor_tensor(out=S2, in0=S2, in1=mview, op=AL.subtract)
                    # E = exp(u) -> bf16 overlay
                    nc.scalar.activation(Ebig[:, :, d0:d0 + P2C], S2, AF.Exp)
                    # Z = ones^T E (before masking, as reference)
                    zp = psz.tile([1, P2C], F32, tag="zp")
                    for j in range(J):
                        nc.tensor.matmul(zp, ones16, Ebig[:, j, d0:d0 + P2C],
                                         start=(j == 0), stop=(j == J - 1))
                    Zrow = work.tile([1, P2C], F32, tag="Zrow")
                rzrow = work.tile([1, P2C], F32, tag="rzrow")
                rzb = work.tile([128, P2C], F32, tag="rzb")
                nc.scalar.activation(Zrow[:, 0:P2C], zp, AF.Copy)
                    # E *= mask rows; E *= 0.25/Z (fold recon normalization)
                    nc.vector.tensor_tensor(out=Ebig[:, :, d0:d0 + P2C],
                                            in0=Ebig[:, :, d0:d0 + P2C], in1=mbv, op=AL.mult)
                    nc.vector.reciprocal(rzrow[:, d0:d0 + P2C], Zrow[:, d0:d0 + P2C])
                    nc.vector.tensor_scalar_mul(rzrow[:, d0:d0 + P2C], rzrow[:, d0:d0 + P2C], 0.25)
                    nc.gpsimd.partition_broadcast(rzb[:, d0:d0 + P2C], rzrow[:, d0:d0 + P2C])
                    rzv = bass.AP(tensor=rzb.tensor, offset=rzb.offset + d0,
                                  ap=[rzb.ap[0], [0, J], [1, P2C]])
                    nc.vector.tensor_tensor(out=Ebig[:, :, d0:d0 + P2C],
                                            in0=Ebig[:, :, d0:d0 + P2C], in1=rzv, op=AL.mult)
        # E gates (phantom att cols)
        nc.vector.tensor_scalar_mul(Ebig[:, :, 0:G], Ebig[:, :, 0:G], gatet[:, 0:1])
        nc.vector.tensor_scalar_mul(Ebig[:, :, NA - G:NA], Ebig[:, :, NA - G:NA], gatet[:, 1:2])

        # ---------------- recon: 4 output-parity classes ----------------
        # out(y=2*yi+ey, x=2*xi+ex) = sum over 4 taps (ky,kx) of class:
        #   raw[tap]^T E'[d], d = (yi+piofs-2)*48 + xi + uofs
        out_acc = big.tile([128, 48, 96], F32, tag="U1")
        with tc.tile_pool(name="psg", bufs=6, space="PSUM") as psg, \
                tc.tile_pool(name="rawp", bufs=2) as rawp:
            for ey in (0, 1):
                for ex in (0, 1):
                    kys = (1, 3) if ey == 0 else (2, 0)
                    kxs = (1, 3) if ex == 0 else (2, 0)
                    # (ky, kx, piofs, uofs); full-width (uofs=0) taps first
                    taps = []
                    for kx in kxs:
                        for ky in kys:
                            piofs = 3 - (ky - 1 - ey) // 2
                            uofs = -(kx - 1 - ex) // 2
                            taps.append((ky, kx, piofs, uofs))
                    raw4 = rawp.tile([128, 4, J, 128], BF16, tag="r4")
                    for ti, (ky, kx, _, _) in enumerate(taps):
                        tap = ky * 4 + kx
                        rin = bass.AP(tensor=rawt_d.ap().tensor, offset=tap * J * 128 * 128,
                                      ap=[[128, 128], [128 * 128, J], [1, 128]])
                        nc.scalar.dma_start(out=raw4[:, ti, :, :], in_=rin)
                    for rc in range(3):
                        yi0 = 8 * rc
                        gp = psg.tile([128, 384], F32, tag="gp")
                        nmm = 0
                        for j in range(J):
                            for ti, (ky, kx, piofs, uofs) in enumerate(taps):
                                rowbase = (yi0 + piofs - 2) * G
                                if uofs == 0:
                                    rhs = Ebig[:, j, rowbase:rowbase + 384]
                                    outv = gp[:, 0:384]
                                else:
                                    xi_lo = 1 if uofs == -1 else 0
                                    rhs = bass.AP(tensor=Ebig.tensor,
                                                  offset=Ebig.offset + j * NCOL + rowbase + xi_lo + uofs,
                                                  ap=[Ebig.ap[0], [G, 8], [1, 47]])
                                    outv = bass.AP(tensor=gp.tensor, offset=gp.offset + xi_lo,
                                                   ap=[gp.ap[0], [G, 8], [1, 47]])
                                nmm += 1
                                nc.tensor.matmul(outv, raw4[:, ti, j, :], rhs,
                                                 start=(nmm == 1), stop=(nmm == 4 * J))
                        oview = bass.AP(tensor=out_acc.tensor,
                                        offset=out_acc.offset + (2 * yi0 + ey) * 96 + ex,
                                        ap=[out_acc.ap[0], [192, 8], [2, 48]])
                        nc.scalar.activation(oview, gp[:, 0:384], AF.Copy)
        nc.sync.dma_start(out=out_d[:, :, :], in_=out_acc)
    nc.finalize()
    return nc


# ======================= host side =======================

def _shift_mats():
    """11 [128,128] fp16 shift matrices (lhsT: out[m] = sum_k M[k,m]*rhs[k])."""
    e = np.eye
    Cw = e(128, k=79); Cw[:, 79] = 0; Cw[:, 127] = 0
    Cpw = e(128, k=-79); Cpw[:, 0] = 0; Cpw[:, 48] = 0
    mats = [e(128), e(128, k=-1), None, e(128, k=1), None,
            e(128, k=-48), e(128, k=80), Cw, e(128, k=48), e(128, k=-80), Cpw]
    Bp1 = np.zeros((128, 128)); Bp1[0, 127] = 1.0
    Bm1 = np.zeros((128, 128)); Bm1[127, 0] = 1.0
    mats[2], mats[4] = Bp1, Bm1
    return np.ascontiguousarray(np.stack(mats, axis=1)).astype(np.float16)


def prep_core_inputs(f, b, mask):
    """Full inputs -> list of 8 in_map dicts (core = 2*s + q)."""
    B = f.shape[0]
    shm = _shift_mats()
    ms = np.pad(mask[0][:, ::8, ::8][0], 1)
    w = np.lib.stride_tricks.sliding_window_view(ms, (3, 3))
    mm = (w.sum((2, 3)) == 0).astype(np.float32).reshape(LB)
    s10 = np.ascontiguousarray((10.0 * mm).reshape(J, 128).T)
    mbin = np.ascontiguousarray(mm.reshape(J, 128).T)
    in_maps = []
    for s in range(B):
        fs = f[s][:, ::2, ::2]
        bs = b[s][:, ::2, ::2]
        fsp = np.pad(fs, ((0, 0), (1, 1), (1, 1)))
        bsp = np.pad(bs, ((0, 0), (1, 1), (1, 1)))
        bhwc = np.pad(b[s], ((0, 0), (1, 1), (1, 1))).transpose(1, 2, 0)
        Q = (bsp.astype(np.float32) ** 2).sum(0).reshape(2500)
        A = Q.copy(); A[0:2499] += Q[1:2500]; A[1:2500] += Q[0:2499]
        Bb = A.copy(); Bb[0:2450] += A[50:2500]; Bb[50:2500] += A[0:2450]
        win = np.lib.stride_tricks.as_strided(
            Bb[51:], shape=(48, 48), strides=(Bb.strides[0] * 50, Bb.strides[0]))
        rd = (1.0 / np.sqrt(win.reshape(LB) + ESC_BIAS)).astype(np.float32)
        rden = np.ascontiguousarray(rd.reshape(J, 128).T)
        wt = np.empty((KT, C, LB), np.float32)
        for o in range(KT):
            dy, dx = o // 3, o % 3
            wt[o] = bsp[:, dy:dy + G, dx:dx + G].reshape(C, LB)
        wt_blocks = np.ascontiguousarray(
            wt.reshape(KT, C, J, 128).transpose(2, 0, 1, 3)).astype(np.float16)
        iy, ix = np.divmod(np.arange(LB), G)
        rawt = np.empty((16, LB, C), np.float32)
        for ky in range(4):
            for kx in range(4):
                rawt[ky * 4 + kx] = bhwc[2 * iy + ky, 2 * ix + kx, :]
        rawt = np.ascontiguousarray(rawt.reshape(16, J, 128, C)).astype(ml_dtypes.bfloat16)
        for q in (0, 1):
            ts_ = np.arange(WINP) - 3 + 24 * q
            fcols = np.zeros((KT, C, NCOL), np.float32)
            valid = (ts_ >= 0) & (ts_ < G)
            for o in range(KT):
                dy, dx = o // 3, o % 3
                block = fsp[:, (ts_ + dy).clip(0, G + 1), :][:, :, dx:dx + G]
                block = block * valid[None, :, None]
                fcols[o, :, :WIN] = block.reshape(C, WIN)
                if q == 1:
                    fcols[o, :, FT0:FT0 + 96] = fsp[:, dy:dy + 2, dx:dx + G].reshape(C, 96)
                else:
                    fcols[o, :, FB0:FB0 + 96] = fsp[:, 46 + dy:48 + dy, dx:dx + G].reshape(C, 96)
            gate = np.zeros((128, 2), np.float32)
            gate[:, 0] = 0.0 if q == 0 else 1.0
            gate[:, 1] = 1.0 if q == 0 else 0.0
            in_maps.append(dict(
                fp=fcols.astype(np.float16),
                wt=wt_blocks,
                rden=rden, shm=shm,
                rawt=rawt,
                s10=s10, mbin=mbin, gate=gate,
            ))
    return in_maps


def assemble(results, B=4):
    out = np.zeros((B, C, 96, 96), np.float32)
    for s in range(B):
        for q in (0, 1):
            out[s, :, 48 * q:48 * q + 48, :] = results[2 * s + q]["out"]
    return out


# ======================= self-contained runner =======================
_NC_CACHE = {}
last_exec_time_ns = None


def kernel(f, b, mask):
    global last_exec_time_ns
    import os
    from concourse.bass_utils import run_bass_kernel_spmd
    f = np.ascontiguousarray(np.asarray(f, dtype=np.float32))
    b = np.ascontiguousarray(np.asarray(b, dtype=np.float32))
    mask = np.ascontiguousarray(np.asarray(mask, dtype=np.float32))
    in_maps = prep_core_inputs(f, b, mask)
    if "nc" not in _NC_CACHE:
        _NC_CACHE["nc"] = build(debug=False)
    nc = _NC_CACHE["nc"]
    trace = bool(os.environ.get("BASS_TRACE"))
    tmpdir = os.environ.get("BASS_TMPDIR") or None
    res = run_bass_kernel_spmd(nc, in_maps, core_ids=list(range(8)), trace=trace,
                               tmpdir=tmpdir)
    last_exec_time_ns = res.exec_time_ns
    return assemble([res.results[i] for i in range(8)], B=f.shape[0])


# revision 4
# speedup vs baseline: 2.5694x; 1.2015x over previous
"""Bass kernel v2 for nn_ContextualAttention on 8 trn2 cores.

Sharding: core = 2*s + q (s = sample 0..3, q = lf-half 0..1), as baseline.
Changes vs baseline:
- pass-1/pass-2 diagonal-fuse moved off the tensor engine: partition-shifted
  SBUF->SBUF DMA snapshots + DVE adds (saves ~116us of PE time).
- recon restructured into 4 output-parity classes: the 4 taps of a class
  accumulate in PSUM (rhs column shifts on E), one strided drain per chunk;
  removes the serialized gpsimd accumulate chain and ~8% of recon rows.
- 1/Z (and /4) folded into E before recon matmuls.
- initial DMAs reordered (fpt on scalar queue, wt first on sync) to cut the
  ~25us start gap.
"""
import numpy as np
import ml_dtypes
import contextlib
import concourse.bass as bass
from concourse import bacc, bass_isa
import concourse.tile as tile
from concourse import mybir

F16 = mybir.dt.float16
F32 = mybir.dt.float32
BF16 = mybir.dt.bfloat16
AL = mybir.AluOpType
AF = mybir.ActivationFunctionType

G = 48
J = 18
KT = 9
LB = 2304
C = 128
WINP = 30
WIN = WINP * G          # 1440
FT0 = WIN               # slab col of far_top block
FB0 = WIN + 96          # slab col of far_bot block
NCOL = WIN + 192        # 1632
CONS0 = 2 * G           # 96
NA = 26 * G             # 1248
SCH = 408               # scores matmul N-chunk (4 per NCOL)
P1W = 192               # pass-1 window chunk (7 per 1344)
P2C = 156               # pass-2 chunk (8 per NA)
ESC_BIAS = 1152 * 1e-4


def build(debug=False):
    nc = bacc.Bacc()
    fp_d = nc.dram_tensor("fp", [KT, 128, NCOL], F16, kind="ExternalInput")
    wt_d = nc.dram_tensor("wt", [J, KT, 128, 128], F16, kind="ExternalInput")
    rden_d = nc.dram_tensor("rden", [128, J], F32, kind="ExternalInput")
    shm_d = nc.dram_tensor("shm", [128, 11, 128], F16, kind="ExternalInput")
    rawt_d = nc.dram_tensor("rawt", [16, J, 128, 128], BF16, kind="ExternalInput")
    s10_d = nc.dram_tensor("s10", [128, J], F32, kind="ExternalInput")
    mbin_d = nc.dram_tensor("mbin", [128, J], F32, kind="ExternalInput")
    gate_d = nc.dram_tensor("gate", [128, 2], F32, kind="ExternalInput")
    out_d = nc.dram_tensor("out", [128, 48, 96], F32, kind="ExternalOutput")
    # DRAM scratch for partition-shifted fuse reads: row r = 1 + lb (lb = j*128+p),
    # rows 0 and 2305 stay zero (shift chain boundaries).
    dsc_d = nc.dram_tensor("dscr", [2306, NCOL], F16, kind="Internal")

    with tile.TileContext(nc) as tc, contextlib.ExitStack() as ctx:
        consts = ctx.enter_context(tc.tile_pool(name="consts", bufs=1))
        wtp = ctx.enter_context(tc.tile_pool(name="wtp", bufs=4))
        big = ctx.enter_context(tc.tile_pool(name="big", bufs=1))
        work = ctx.enter_context(tc.tile_pool(name="work", bufs=2))

        # ---------------- consts (gpsimd queue; tiny) ----------------
        s10t = consts.tile([128, J], F32, tag="s10t")
        nc.gpsimd.dma_start(out=s10t, in_=s10_d[:, :])
        mbint = consts.tile([128, J], F32, tag="mbint")
        nc.gpsimd.dma_start(out=mbint, in_=mbin_d[:, :])
        gatet = consts.tile([128, 2], F32, tag="gatet")
        nc.gpsimd.dma_start(out=gatet, in_=gate_d[:, :])
        rdent = consts.tile([128, J], F32, tag="rdent")
        nc.gpsimd.dma_start(out=rdent, in_=rden_d[:, :])
        ones16 = consts.tile([128, 1], BF16, tag="ones16")
        nc.vector.memset(ones16, 1.0)
        zc = consts.tile([128, NCOL], F16, tag="zc")
        nc.vector.memset(zc, 0.0)
        nc.gpsimd.dma_start(out=dsc_d[0:1, :], in_=zc[0:1, :])
        nc.gpsimd.dma_start(out=dsc_d[2305:2306, :], in_=zc[0:1, :])
        # shift matrices for pass-2 matmuls:
        # [I, ., ., ., ., Ap48, Bp48, Cw, Am48, Bm48, Cpw]
        shmt = consts.tile([128, 11, 128], F16, tag="shmt")
        nc.gpsimd.dma_start(out=shmt, in_=shm_d[:, :, :])

        # ---------------- scores GEMM -> slab (Sn, fp16) ----------------
        # wt j=0 goes first on sync; fpt streams on scalar queue.
        fpt = big.tile([128, KT, NCOL], F16, tag="U1")
        slab = big.tile([128, J, NCOL], F16, tag="slab")
        engs = [nc.sync, nc.scalar, nc.gpsimd]
        for o in range(KT):
            engs[o % 3].dma_start(out=fpt[:, o, :], in_=fp_d[o, :, :])
        with tc.tile_pool(name="psc", bufs=2, space="PSUM") as psc:
            for j in range(J):
                ps = psc.tile([128, 4, 512], F32, tag="sps")
                for o in range(KT):
                    wb = wtp.tile([128, 128], F16, tag="wb")
                    eng = nc.sync if (o % 2 == 0) else nc.gpsimd
                    eng.dma_start(out=wb, in_=wt_d[j, o, :, :])
                    for ch in range(4):
                        nc.tensor.matmul(ps[:, ch, 0:SCH], wb,
                                         fpt[:, o, ch * SCH:(ch + 1) * SCH],
                                         start=(o == 0), stop=(o == KT - 1))
                for ch in range(4):
                    nc.scalar.activation(slab[:, j, ch * SCH:(ch + 1) * SCH],
                                         ps[:, ch, 0:SCH], AF.Copy,
                                         scale=rdent[:, j:j + 1])
                engs[j % 3].dma_start(out=dsc_d[1 + j * 128:1 + (j + 1) * 128, :],
                                      in_=slab[:, j, :])

        # ---------------- pass-1: S1 = Sn + diag(+1) + diag(-1), on DVE -------
        # S1 layout as baseline: local l = global-48 for window, far_top at
        # [1344,1440) <- slab[1440,1536), far_bot [1440,1536) <- slab[1536,1632).
        # Partition shifts via full-width SBUF->SBUF DMA snapshots, one j-half
        # at a time (per-partition contiguous runs -> 1 descriptor each).
        S1 = big.tile([128, J, 1536], F16, tag="U1")
        with tc.tile_pool(name="shift", bufs=2) as shp:
            for gi, js in enumerate(range(0, J, 3)):
                je = js + 3
                nj = je - js
                Ah = shp.tile([128, 3, NCOL], F16, tag="shA")
                ain = bass.AP(tensor=dsc_d.ap().tensor, offset=(js * 128 + 2) * NCOL,
                              ap=[[NCOL, 128], [128 * NCOL, nj], [1, NCOL]])
                engs[gi % 3].dma_start(out=Ah[:, 0:nj, :], in_=ain)
                Mh = shp.tile([128, 3, NCOL], F16, tag="shM")
                min_ = bass.AP(tensor=dsc_d.ap().tensor, offset=js * 128 * NCOL,
                               ap=[[NCOL, 128], [128 * NCOL, nj], [1, NCOL]])
                engs[(gi + 1) % 3].dma_start(out=Mh[:, 0:nj, :], in_=min_)
                nc.vector.tensor_tensor(out=S1[:, js:je, 0:1344],
                                        in0=slab[:, js:je, 48:1392],
                                        in1=Ah[:, 0:nj, 49:1393], op=AL.add)
                nc.vector.tensor_tensor(out=S1[:, js:je, 0:1344],
                                        in0=S1[:, js:je, 0:1344],
                                        in1=Mh[:, 0:nj, 47:1391], op=AL.add)
                for f0, l0 in ((FT0, 1344), (FB0, 1440)):
                    nc.scalar.activation(S1[:, js:je, l0:l0 + 96],
                                         slab[:, js:je, f0:f0 + 96], AF.Copy)
                    nc.vector.tensor_tensor(out=S1[:, js:je, l0:l0 + 95],
                                            in0=S1[:, js:je, l0:l0 + 95],
                                            in1=Ah[:, 0:nj, f0 + 1:f0 + 96], op=AL.add)
                    nc.vector.tensor_tensor(out=S1[:, js:je, l0 + 1:l0 + 96],
                                            in0=S1[:, js:je, l0 + 1:l0 + 96],
                                            in1=Mh[:, 0:nj, f0:f0 + 95], op=AL.add)
            # gates: zero pi=2 block (q=0) / pi=27 block (q=1)
            nc.vector.tensor_scalar_mul(S1[:, :, 48:96], S1[:, :, 48:96], gatet[:, 0:1])
            nc.vector.tensor_scalar_mul(S1[:, :, 1248:1296], S1[:, :, 1248:1296], gatet[:, 1:2])

        # ---------------- pass-2 (shift matmuls) + softmax (E overlays slab) --
        SH_I = 0
        SH_AP48, SH_BP48, SH_CW, SH_AM48, SH_BM48, SH_CPW = 5, 6, 7, 8, 9, 10
        b_lo, b_hi = 26 * G, 26 * G + G - 1
        bp_lo, bp_hi = 3 * G + 1, 3 * G + G
        Ebig = slab[:, :, 0:NA].bitcast(BF16)
        mbv = bass.AP(tensor=mbint.tensor, offset=mbint.offset,
                      ap=[mbint.ap[0], [1, J], [0, P2C]])
        with tc.tile_pool(name="psp2", bufs=6, space="PSUM") as psp2, \
                tc.tile_pool(name="psz", bufs=2, space="PSUM") as psz:
            for ci in range(NA // P2C):
                d0 = ci * P2C
                g0 = CONS0 + d0
                l0 = g0 - 96
                S2 = work.tile([128, J, P2C], F32, tag="S2")
                for j in range(J):
                    pq = psp2.tile([128, P2C], F32, tag="pq")
                    nc.tensor.matmul(pq, shmt[:, SH_I, :], S1[:, j, l0 + 48:l0 + 48 + P2C],
                                     start=True, stop=False)
                    nc.tensor.matmul(pq, shmt[:, SH_AP48, :], S1[:, j, l0 + 96:l0 + 96 + P2C],
                                     start=False, stop=False)
                    nc.tensor.matmul(pq, shmt[:, SH_AM48, :], S1[:, j, l0:l0 + P2C],
                                     start=False, stop=False)
                    if j < J - 1:
                        nc.tensor.matmul(pq, shmt[:, SH_BP48, :], S1[:, j + 1, l0 + 96:l0 + 96 + P2C],
                                         start=False, stop=False)
                    else:
                        nc.tensor.matmul(pq, shmt[:, SH_CW, :], S1[:, 0, l0 + 96:l0 + 96 + P2C],
                                         start=False, stop=False)
                    last_main = (not (b_lo < g0 + P2C and b_hi > g0)) and (not (bp_lo < g0 + P2C and bp_hi > g0))
                    if j > 0:
                        nc.tensor.matmul(pq, shmt[:, SH_BM48, :], S1[:, j - 1, l0:l0 + P2C],
                                         start=False, stop=last_main)
                    else:
                        nc.tensor.matmul(pq, shmt[:, SH_CPW, :], S1[:, 17, l0:l0 + P2C],
                                         start=False, stop=last_main)
                    if b_lo < g0 + P2C and b_hi > g0:
                        aa = max(b_lo, g0); bb = min(b_hi, g0 + P2C)
                        so = aa - b_lo
                        ft = 1344 + 1 + so
                        oa, nb = aa - g0, bb - aa
                        nc.tensor.matmul(pq[:, oa:oa + nb], shmt[:, SH_AP48, :], S1[:, j, ft:ft + nb],
                                         start=False, stop=False)
                        if j < J - 1:
                            nc.tensor.matmul(pq[:, oa:oa + nb], shmt[:, SH_BP48, :], S1[:, j + 1, ft:ft + nb],
                                             start=False, stop=True)
                        else:
                            nc.tensor.matmul(pq[:, oa:oa + nb], shmt[:, SH_CW, :], S1[:, 0, ft:ft + nb],
                                             start=False, stop=True)
                    if bp_lo < g0 + P2C and bp_hi > g0:
                        aa = max(bp_lo, g0); bb = min(bp_hi, g0 + P2C)
                        so = aa - bp_lo
                        fb = 1440 + 48 + so
                        oa, nb = aa - g0, bb - aa
                        nc.tensor.matmul(pq[:, oa:oa + nb], shmt[:, SH_AM48, :], S1[:, j, fb:fb + nb],
                                         start=False, stop=False)
                        if j > 0:
                            nc.tensor.matmul(pq[:, oa:oa + nb], shmt[:, SH_BM48, :], S1[:, j - 1, fb:fb + nb],
                                             start=False, stop=True)
                        else:
                            nc.tensor.matmul(pq[:, oa:oa + nb], shmt[:, SH_CPW, :], S1[:, 17, fb:fb + nb],
                                             start=False, stop=True)
                    # drain: S2 = pq * s10 (scale applied by activation)
                    nc.scalar.activation(S2[:, j, :], pq, AF.Copy, scale=s10t[:, j:j + 1])
                    # max over lb
                    t9 = work.tile([128, 9, P2C], BF16, tag="tA")
                    nc.vector.tensor_tensor(out=t9, in0=S2[:, 0:9, :], in1=S2[:, 9:18, :], op=AL.max)
                    t4 = work.tile([128, 4, P2C], BF16, tag="tB")
                    nc.vector.tensor_tensor(out=t4, in0=t9[:, 0:4, :], in1=t9[:, 4:8, :], op=AL.max)
                    t2 = work.tile([128, 2, P2C], BF16, tag="t2")
                    nc.vector.tensor_tensor(out=t2, in0=t4[:, 0:2, :], in1=t4[:, 2:4, :], op=AL.max)
                    mx = work.tile([128, P2C], BF16, tag="mx")
                    nc.vector.tensor_tensor(out=mx, in0=t2[:, 0, :], in1=t2[:, 1, :], op=AL.max)
                    nc.vector.tensor_tensor(out=mx, in0=mx, in1=t9[:, 8, :], op=AL.max)
                    mxb = work.tile([128, P2C], F32, tag="mxb")
                    nc.gpsimd.partition_all_reduce(mxb, mx, channels=128,
                                                   reduce_op=bass_isa.ReduceOp.max)
                    mview = bass.AP(tensor=mxb.tensor, offset=mxb.offset,
                                    ap=[mxb.ap[0], [0, J], mxb.ap[1]])
                    nc.vector.tensor_tensor(out=S2, in0=S2, in1=mview, op=AL.subtract)
                    # E = exp(u) -> bf16 overlay
                    nc.scalar.activation(Ebig[:, :, d0:d0 + P2C], S2, AF.Exp)
                    # Z = ones^T E (before masking, as reference)
                    zp = psz.tile([1, P2C], F32, tag="zp")
                    for j in range(J):
                        nc.tensor.matmul(zp, ones16, Ebig[:, j, d0:d0 + P2C],
                                         start=(j == 0), stop=(j == J - 1))
                    Zrow = work.tile([1, P2C], F32, tag="Zrow")
                rzrow = work.tile([1, P2C], F32, tag="rzrow")
                rzb = work.tile([128, P2C], F32, tag="rzb")
                nc.scalar.activation(Zrow[:, 0:P2C], zp, AF.Copy)
                    # E *= mask rows; E *= 0.25/Z (fold recon normalization)
                    nc.vector.tensor_tensor(out=Ebig[:, :, d0:d0 + P2C],
                                            in0=Ebig[:, :, d0:d0 + P2C], in1=mbv, op=AL.mult)
                    nc.vector.reciprocal(rzrow[:, d0:d0 + P2C], Zrow[:, d0:d0 + P2C])
                    nc.vector.tensor_scalar_mul(rzrow[:, d0:d0 + P2C], rzrow[:, d0:d0 + P2C], 0.25)
                    nc.gpsimd.partition_broadcast(rzb[:, d0:d0 + P2C], rzrow[:, d0:d0 + P2C])
                    rzv = bass.AP(tensor=rzb.tensor, offset=rzb.offset + d0,
                                  ap=[rzb.ap[0], [0, J], [1, P2C]])
                    nc.vector.tensor_tensor(out=Ebig[:, :, d0:d0 + P2C],
                                            in0=Ebig[:, :, d0:d0 + P2C], in1=rzv, op=AL.mult)
        # E gates (phantom att cols)
        nc.vector.tensor_scalar_mul(Ebig[:, :, 0:G], Ebig[:, :, 0:G], gatet[:, 0:1])
        nc.vector.tensor_scalar_mul(Ebig[:, :, NA - G:NA], Ebig[:, :, NA - G:NA], gatet[:, 1:2])

        # ---------------- recon: 4 output-parity classes ----------------
        # out(y=2*yi+ey, x=2*xi+ex) = sum over 4 taps (ky,kx) of class:
        #   raw[tap]^T E'[d], d = (yi+piofs-2)*48 + xi + uofs
        out_acc = big.tile([128, 48, 96], F32, tag="U1")
        with tc.tile_pool(name="psg", bufs=6, space="PSUM") as psg, \
                tc.tile_pool(name="rawp", bufs=2) as rawp:
            for ey in (0, 1):
                for ex in (0, 1):
                    kys = (1, 3) if ey == 0 else (2, 0)
                    kxs = (1, 3) if ex == 0 else (2, 0)
                    # (ky, kx, piofs, uofs); full-width (uofs=0) taps first
                    taps = []
                    for kx in kxs:
                        for ky in kys:
                            piofs = 3 - (ky - 1 - ey) // 2
                            uofs = -(kx - 1 - ex) // 2
                            taps.append((ky, kx, piofs, uofs))
                    raw4 = rawp.tile([128, 4, J, 128], BF16, tag="r4")
                    for ti, (ky, kx, _, _) in enumerate(taps):
                        tap = ky * 4 + kx
                        rin = bass.AP(tensor=rawt_d.ap().tensor, offset=tap * J * 128 * 128,
                                      ap=[[128, 128], [128 * 128, J], [1, 128]])
                        engs[ti % 3].dma_start(out=raw4[:, ti, :, :], in_=rin)
                    for rc in range(3):
                        yi0 = 8 * rc
                        gp = psg.tile([128, 384], F32, tag="gp")
                        nmm = 0
                        for j in range(J):
                            for ti, (ky, kx, piofs, uofs) in enumerate(taps):
                                rowbase = (yi0 + piofs - 2) * G
                                if uofs == 0:
                                    rhs = Ebig[:, j, rowbase:rowbase + 384]
                                    outv = gp[:, 0:384]
                                else:
                                    xi_lo = 1 if uofs == -1 else 0
                                    rhs = bass.AP(tensor=Ebig.tensor,
                                                  offset=Ebig.offset + j * NCOL + rowbase + xi_lo + uofs,
                                                  ap=[Ebig.ap[0], [G, 8], [1, 47]])
                                    outv = bass.AP(tensor=gp.tensor, offset=gp.offset + xi_lo,
                                                   ap=[gp.ap[0], [G, 8], [1, 47]])
                                nmm += 1
                                nc.tensor.matmul(outv, raw4[:, ti, j, :], rhs,
                                                 start=(nmm == 1), stop=(nmm == 4 * J))
                        oview = bass.AP(tensor=out_acc.tensor,
                                        offset=out_acc.offset + (2 * yi0 + ey) * 96 + ex,
                                        ap=[out_acc.ap[0], [192, 8], [2, 48]])
                        nc.scalar.activation(oview, gp[:, 0:384], AF.Copy)
        nc.sync.dma_start(out=out_d[:, :, :], in_=out_acc)
    nc.finalize()
    return nc


# ======================= host side =======================

def _shift_mats():
    """11 [128,128] fp16 shift matrices (lhsT: out[m] = sum_k M[k,m]*rhs[k])."""
    e = np.eye
    Cw = e(128, k=79); Cw[:, 79] = 0; Cw[:, 127] = 0
    Cpw = e(128, k=-79); Cpw[:, 0] = 0; Cpw[:, 48] = 0
    mats = [e(128), e(128, k=-1), None, e(128, k=1), None,
            e(128, k=-48), e(128, k=80), Cw, e(128, k=48), e(128, k=-80), Cpw]
    Bp1 = np.zeros((128, 128)); Bp1[0, 127] = 1.0
    Bm1 = np.zeros((128, 128)); Bm1[127, 0] = 1.0
    mats[2], mats[4] = Bp1, Bm1
    return np.ascontiguousarray(np.stack(mats, axis=1)).astype(np.float16)


def prep_core_inputs(f, b, mask):
    """Full inputs -> list of 8 in_map dicts (core = 2*s + q)."""
    B = f.shape[0]
    shm = _shift_mats()
    ms = np.pad(mask[0][:, ::8, ::8][0], 1)
    w = np.lib.stride_tricks.sliding_window_view(ms, (3, 3))
    mm = (w.sum((2, 3)) == 0).astype(np.float32).reshape(LB)
    s10 = np.ascontiguousarray((10.0 * mm).reshape(J, 128).T)
    mbin = np.ascontiguousarray(mm.reshape(J, 128).T)
    in_maps = []
    for s in range(B):
        fs = f[s][:, ::2, ::2]
        bs = b[s][:, ::2, ::2]
        fsp = np.pad(fs, ((0, 0), (1, 1), (1, 1)))
        bsp = np.pad(bs, ((0, 0), (1, 1), (1, 1)))
        bhwc = np.pad(b[s], ((0, 0), (1, 1), (1, 1))).transpose(1, 2, 0)
        Q = (bsp.astype(np.float32) ** 2).sum(0).reshape(2500)
        A = Q.copy(); A[0:2499] += Q[1:2500]; A[1:2500] += Q[0:2499]
        Bb = A.copy(); Bb[0:2450] += A[50:2500]; Bb[50:2500] += A[0:2450]
        win = np.lib.stride_tricks.as_strided(
            Bb[51:], shape=(48, 48), strides=(Bb.strides[0] * 50, Bb.strides[0]))
        rd = (1.0 / np.sqrt(win.reshape(LB) + ESC_BIAS)).astype(np.float32)
        rden = np.ascontiguousarray(rd.reshape(J, 128).T)
        wt = np.empty((KT, C, LB), np.float32)
        for o in range(KT):
            dy, dx = o // 3, o % 3
            wt[o] = bsp[:, dy:dy + G, dx:dx + G].reshape(C, LB)
        wt_blocks = np.ascontiguousarray(
            wt.reshape(KT, C, J, 128).transpose(2, 0, 1, 3)).astype(np.float16)
        iy, ix = np.divmod(np.arange(LB), G)
        rawt = np.empty((16, LB, C), np.float32)
        for ky in range(4):
            for kx in range(4):
                rawt[ky * 4 + kx] = bhwc[2 * iy + ky, 2 * ix + kx, :]
        rawt = np.ascontiguousarray(rawt.reshape(16, J, 128, C)).astype(ml_dtypes.bfloat16)
        for q in (0, 1):
            ts_ = np.arange(WINP) - 3 + 24 * q
            fcols = np.zeros((KT, C, NCOL), np.float32)
            valid = (ts_ >= 0) & (ts_ < G)
            for o in range(KT):
                dy, dx = o // 3, o % 3
                block = fsp[:, (ts_ + dy).clip(0, G + 1), :][:, :, dx:dx + G]
                block = block * valid[None, :, None]
                fcols[o, :, :WIN] = block.reshape(C, WIN)
                if q == 1:
                    fcols[o, :, FT0:FT0 + 96] = fsp[:, dy:dy + 2, dx:dx + G].reshape(C, 96)
                else:
                    fcols[o, :, FB0:FB0 + 96] = fsp[:, 46 + dy:48 + dy, dx:dx + G].reshape(C, 96)
            gate = np.zeros((128, 2), np.float32)
            gate[:, 0] = 0.0 if q == 0 else 1.0
            gate[:, 1] = 1.0 if q == 0 else 0.0
            in_maps.append(dict(
                fp=fcols.astype(np.float16),
                wt=wt_blocks,
                rden=rden, shm=shm,
                rawt=rawt,
                s10=s10, mbin=mbin, gate=gate,
            ))
    return in_maps


def assemble(results, B=4):
    out = np.zeros((B, C, 96, 96), np.float32)
    for s in range(B):
        for q in (0, 1):
            out[s, :, 48 * q:48 * q + 48, :] = results[2 * s + q]["out"]
    return out


# ======================= self-contained runner =======================
_NC_CACHE = {}
last_exec_time_ns = None


def kernel(f, b, mask):
    global last_exec_time_ns
    import os
    from concourse.bass_utils import run_bass_kernel_spmd
    f = np.ascontiguousarray(np.asarray(f, dtype=np.float32))
    b = np.ascontiguousarray(np.asarray(b, dtype=np.float32))
    mask = np.ascontiguousarray(np.asarray(mask, dtype=np.float32))
    in_maps = prep_core_inputs(f, b, mask)
    if "nc" not in _NC_CACHE:
        _NC_CACHE["nc"] = build(debug=False)
    nc = _NC_CACHE["nc"]
    trace = bool(os.environ.get("BASS_TRACE"))
    tmpdir = os.environ.get("BASS_TMPDIR") or None
    res = run_bass_kernel_spmd(nc, in_maps, core_ids=list(range(8)), trace=trace,
                               tmpdir=tmpdir)
    last_exec_time_ns = res.exec_time_ns
    return assemble([res.results[i] for i in range(8)], B=f.shape[0])
